# revision 89
# baseline (speedup 1.0000x reference)
"""8-core Trainium2 Bass kernel for nn_BolmoLocalLayer (v7).

Strategy (uniform SPMD program, rank-dependence only in data):
 - host: fold norm1 into Wcat/wv, mh_w into w_out, norm2 into w_gate/w_up;
   pre-transpose x per core; pre-cast weights to bf16 AND pre-tile every
   weight into the exact [partition, kt, col] SBUF layout so each weight
   DMA is one fully-contiguous 4KB+/partition stream (strided 256B-line
   gathers ran at ~1/4 bandwidth and starved the PE).
 - token-parallel projections (each core: its 512 tokens, all heads);
   og projection stays LOCAL (sigmoid applied on token-owner) - no og A2A.
 - AllToAll: core c receives head c's qT/kT/v (+ gates) for all tokens.
 - head-sharded mLSTM attention with the rank-1 decay factorization
   exp(Fcum_t - Fcum_s + i_s) = alpha'[key] * beta'[query] per query-block
   (alpha' = exp(dcol - dmax_u) <= 1 so fp32 FTZ is safe): ONE fused DVE
   op per qk tile; beta' = exp(Fcum + dmax_u) un-shifts the per-query
   scale BEFORE squaring in the per-head RMS-norm tail.  The normalizer
   n = colsum(C) accumulates on the DVE; per-block tails are emitted one
   block late and h_norm is computed from the SBUF copies At = A*beta',
   so the tail chain never blocks the PE or the PSUM A-buffers.
 - per-batch AllToAll of normalized h back to token-owners, with the
   h_out reload issued right behind each collective; batch-major local
   full-D w_out matmul; x1 = x + mix; norm2 -> per-batch AllGather.
 - FF-sharded SwiGLU MLP; chunk 0's gate/up split by batch halves so its
   b0 columns run while the b1 AllGather is still in flight.
 - host: scatter x1 rows + sum MLP partials.
"""
import sys
from contextlib import ExitStack

for _p in ("/opt/trn_rl_repo", "/root/.axon_site/_ro/trn_rl_repo"):
    if _p not in sys.path:
        sys.path.append(_p)

import numpy as np
import ml_dtypes

import concourse.mybir as mybir
from concourse import bacc
from concourse.tile import TileContext
from concourse.bass_utils import run_bass_kernel_spmd
from concourse.bass import ds

bf16 = ml_dtypes.bfloat16
FP32 = mybir.dt.float32
BF16 = mybir.dt.bfloat16

B, S, D, H = 2, 2048, 2048, 8
QK, FF = D // 2, 8192
dqk, dv = QK // H, D // H        # 128, 256
R = 8                            # cores
OB = S // R                      # 256 own tokens per batch
OT = 2 * OB                      # 512 own tokens
NK = D // 128                    # 16 contraction tiles over D
CAP, EPS = 15.0, 1e-6
FFC = FF // R                    # 1024 ff slice per core
NM = 33                          # wcat m-tiles (q8 k8 og16 gates1)
# a2a1 payload (bf16 elems per dest): qT 128x512 | kT 128x512
PAY_Q, PAY_K, PAY1 = 0, 65536, 131072
PAYV = 131072                    # v payload: 4 x (128x256)
PAYH = 65536                     # h payload per batch: 128 x 2 x 256

AL = mybir.AluOpType
AF = mybir.ActivationFunctionType


def _gate_prelude(nc, ag_g_out, G0, Fcum, dcol, irow_g, dscr, ones1f):
    """Build Fcum, dcol[b] (= i - Fcum as [128 key, 16 ck]) and Mneg rows.

    G0 segments (x S): 0 i_b0 | 1 i_b1 | 2 f_b0 | 3 f_b1
    (f transformed in place to logsigmoid; i overwritten by i - Fcum).
    """
    for b in range(2):
        tsl = (slice(0, OB), slice(OB, OT))[b]
        for src in range(R):
            so = OB * (src if b == 0 else 7 - src)
            nc.scalar.dma_start(G0[:, ds(0 * S + so, OB)],
                                ag_g_out[src, 0:1, tsl])
            nc.scalar.dma_start(G0[:, ds(1 * S + so, OB)],
                                ag_g_out[src, 1:2, tsl])
        wk = G0[:, ds(1 * S, S)]
        nc.scalar.activation(wk, wk, AF.Exp, scale=-1.0)
        # ln(1 + e^-f) in one activation (bias folds the +1): one less
        # skew-gated op in the vector queue
        nc.scalar.activation(wk, wk, AF.Ln, bias=ones1f[0:1, 0:1])
        nc.vector.tensor_scalar_mul(wk, wk, -1.0)
        nc.vector.tensor_tensor_scan(Fcum[b][:], wk, wk, 0.0,
                                     AL.add, AL.bypass)
        nc.vector.tensor_tensor(irow_g[b][:], G0[:, ds(0 * S, S)], Fcum[b][:],
                                AL.subtract)
        # dcol[b][p, ck] = (i - Fcum)[ck*128 + p]  (via DRAM scratch --
        # direct SBUF->SBUF partition-scatter DMA produces garbage)
        nc.scalar.dma_start(dscr[b][:], irow_g[b][:])
        nc.scalar.dma_start(dcol[b][:],
                            dscr[b].rearrange("(ck p) -> p ck", p=128))


def _decay_factors(nc, irow_g, dcol, dm, negdm, abias, alpha):
    """dm[b][u] = running max over keys < (u+1)*512 of (i - Fcum);
    alpha[p, b, u, ck] = exp(dcol - dmax_u) <= 1 (decay column factors)."""
    for b in range(2):
        nc.vector.tensor_reduce(
            dm[:, b], irow_g[b][:].rearrange("p (u t) -> p u t", u=4),
            mybir.AxisListType.X, AL.max)
        for u in range(1, 4):
            nc.vector.tensor_tensor(dm[:, b, u:u + 1], dm[:, b, u - 1:u],
                                    dm[:, b, u:u + 1], AL.max)
        nc.vector.tensor_scalar_mul(negdm[:, b], dm[:, b], -1.0)
        for u in range(4):
            nc.gpsimd.partition_broadcast(abias[:, b, u:u + 1],
                                          negdm[:, b, u:u + 1])
    for b in range(2):
        for tb in range(4):
            nst = 4 * tb + 4
            nc.scalar.activation(alpha[:, b, tb, 0:nst], dcol[b][:, 0:nst],
                                 AF.Exp, bias=abias[:, b, tb:tb + 1])


def _mixer(nc, tc, xbs, wcat_d, wv_d, b16, sigog, a2a1_in, a2a1_out, a2av_in,
           a2av_out, ag_g_in, ag_g_out, rg, onesb, ones1f, Fcum, dcol,
           irow_g, dscr, dm, negdm, abias, alpha, qkin):
    """norm1 + projections for own 512 tokens.

    Order: gates first (fires ag_g A2A), then the gate prelude (scalar/
    vector-queue ops wait on ag_g while the tensor engine does q/k), q/k ->
    a2a1, v -> a2av, og last (local; sigmoid applied straight from PSUM).
    """
    with tc.tile_pool(name="mx_w", bufs=6) as wp, \
         tc.tile_pool(name="mx_wv", bufs=2) as wvp, \
         tc.tile_pool(name="mx_tmp", bufs=3) as tp, \
         tc.tile_pool(name="mx_out", bufs=1) as op, \
         tc.tile_pool(name="mx_ps", bufs=3, space="PSUM") as ps, \
         tc.tile_pool(name="mx_ps1", bufs=1, space="PSUM") as ps1:
        G0 = op.tile([1, 2 * S], FP32)
        xhat = op.tile([128, NK, OT], BF16)
        # ssq over D (partition axis) via bf16 squares + ones-matmul;
        # per xb chunk so it interleaves with the chunked x DMA
        ssq_ps = ps1.tile([1, OT], FP32, tag="ssq1")
        for kt in range(NK):
            sq = tp.tile([128, OT], BF16, tag="sq")
            xk = xbs[kt // 4][:, kt % 4]
            nc.vector.tensor_tensor(sq[:], xk, xk, AL.mult)
            nc.tensor.matmul(ssq_ps[:], onesb[:], sq[:],
                             start=(kt == 0), stop=(kt == NK - 1))
        scl = tp.tile([1, OT], FP32, tag="scl1")
        nc.vector.tensor_scalar(scl[:], ssq_ps[:], 1.0 / D, EPS, AL.mult, AL.add)
        nc.scalar.activation(scl[:], scl[:], AF.Ln)
        nc.scalar.activation(scl[:], scl[:], AF.Exp, scale=-0.5)
        sc_ps = ps1.tile([128, OT], FP32, tag="sc1")
        nc.tensor.matmul(sc_ps[:], ones1f[0:1, :], scl[:], start=True, stop=True)
        for kt in range(NK):
            nc.vector.tensor_tensor(xhat[:, kt], xbs[kt // 4][:, kt % 4],
                                    sc_ps[:], AL.mult)

        # --- Wcat projections: out[F, own-t].  M-tiles: 0..7 q | 8..15 k |
        # 16..31 og | 32 gates(16 cols, [i0 f0 i1 f1 ...])
        qT = op.tile([128, H, OT], BF16)
        kT = op.tile([128, H, OT], BF16)
        # "pre" after v: its vector-queue ops wait on the ag_g collective
        # (which absorbs cross-core start skew); the scheduler interleaves
        # them with projection evacuations wherever they're emitted, so
        # they sit where the least tensor work depends on the evacuations
        for m in [32] + list(range(16)) + ["v", "pre"] + list(range(16, 32)):
            if m == "v":
                _v_proj(nc, xhat, wv_d, a2av_in, a2av_out, rg, wvp, tp, ps)
                continue
            if m == "pre":
                _gate_prelude(nc, ag_g_out, G0, Fcum, dcol, irow_g, dscr,
                              ones1f)
                _decay_factors(nc, irow_g, dcol, dm, negdm, abias, alpha)
                continue
            mw = 16 if m == 32 else 128
            wt = wp.tile([128, NK, 128], BF16, tag="wcat")
            nc.sync.dma_start(wt[:], wcat_d[m])
            pst = ps.tile([128, OT], FP32, tag="proj")
            for kt in range(NK):
                nc.tensor.matmul(pst[:mw, :], wt[:, kt, :mw], xhat[:, kt],
                                 start=(kt == 0), stop=(kt == NK - 1))
            if m < 8:
                nc.vector.tensor_scalar_mul(qT[:, m], pst[:], 1.0 / np.sqrt(dqk))
            elif m < 16:
                nc.vector.tensor_copy(kT[:, m - 8], pst[:])
            elif m < 32:
                # sigmoid(og) straight from PSUM; stays local to this core
                nc.scalar.activation(sigog[:, m - 16], pst[:], AF.Sigmoid)
            else:
                gt = tp.tile([16, OT], FP32, tag="gates")
                nc.scalar.activation(gt[:], pst[:16, :], AF.Exp,
                                     bias=b16[:], scale=-2.0 / CAP)
                nc.vector.tensor_scalar_add(gt[:], gt[:], 1.0)
                nc.vector.reciprocal(gt[:], gt[:])
                nc.vector.tensor_scalar(gt[:], gt[:], 2.0 * CAP, -CAP,
                                        AL.mult, AL.add)
                for hh in range(H):
                    nc.sync.dma_start(ag_g_in[hh], gt[ds(2 * hh, 2), :])
                nc.gpsimd.collective_compute(
                    "AllToAll", AL.bypass, replica_groups=rg,
                    ins=[ag_g_in[:]], outs=[ag_g_out[:]])
            if m == 15:
                # q and k projections done -> payloads + early A2A
                for hh in range(H):
                    nc.sync.dma_start(
                        a2a1_in[hh, ds(PAY_Q, 65536)].rearrange(
                            "(p t) -> p t", p=128), qT[:, hh])
                    nc.sync.dma_start(
                        a2a1_in[hh, ds(PAY_K, 65536)].rearrange(
                            "(p t) -> p t", p=128), kT[:, hh])
                nc.gpsimd.collective_compute(
                    "AllToAll", AL.bypass, replica_groups=rg,
                    ins=[a2a1_in[:]], outs=[a2a1_out[:]])
                # q/k attention inputs reload right behind their A2A --
                # before the a2av collective blocks the gpsimd queue
                qT_all, kT_all = qkin
                nc.gpsimd.dma_start(
                    qT_all[:],
                    a2a1_out[:, ds(PAY_Q, 65536)].rearrange(
                        "r (p t) -> p r t", p=128))
                nc.gpsimd.dma_start(
                    kT_all[:],
                    a2a1_out[:, ds(PAY_K, 65536)].rearrange(
                        "r (p t) -> p r t", p=128))


def _v_proj(nc, xhat, wv_d, a2av_in, a2av_out, rg, wvp, tp, ps):
    """v = xhat.T @ wv, natural [own-t, D]; per-head payload + A2A."""
    for nb in range(4):          # D output chunks of 512
        wvt = wvp.tile([128, NK, 512], BF16, tag="wv")
        nc.sync.dma_start(wvt[:], wv_d[nb])
        for tt in range(4):      # own-token tiles of 128
            pst = ps.tile([128, 512], FP32, tag="vproj")
            for kt in range(NK):
                nc.tensor.matmul(pst[:], xhat[:, kt, ds(tt * 128, 128)],
                                 wvt[:, kt], start=(kt == 0), stop=(kt == NK - 1))
            vsb = tp.tile([128, 512], BF16, tag="vsb")
            nc.vector.tensor_copy(vsb[:], pst[:])
            # dest head hh owns cols [hh*256, hh*256+256) of global D;
            # payload is partition-major so the receiver's reload is one
            # 2KB-contiguous stream per partition per source
            for hh in (2 * nb, 2 * nb + 1):
                off = hh * 256 - nb * 512
                nc.sync.dma_start(
                    a2av_in[hh].rearrange("(p tc) -> p tc",
                                          p=128)[:, ds(tt * 256, 256)],
                    vsb[:, ds(off, 256)])
    nc.gpsimd.collective_compute(
        "AllToAll", AL.bypass, replica_groups=rg,
        ins=[a2av_in[:]], outs=[a2av_out[:]])


def _attention(nc, tc, a2a1_out, a2av_out, strip, Fcum, dm, alpha,
               onesb, a2ah_in, a2ah_out, rg, prefetch, houts, qkin):
    """head-c mLSTM over full S for b in {0,1}; A2A h_norm per batch.

    Input loads ride the gpsimd queue so they fire the moment each A2A
    lands; the at_in pool lives on the right-side SBUF stack so the
    loads carry no WAR dependency on the og-phase tiles.  Tails are
    emitted one query-block late; h_norm is computed from the SBUF
    copies At = A*beta', so neither the PSUM A-buffers nor the tensor
    queue ever wait on a tail chain.  Each batch's h reload (houts) is
    issued right behind its a2ah collective.  tp/rp are owned by the
    caller and closed only after post: their release instructions wait
    on the last payload DMAs and would otherwise head-block the vector
    queue right when post's pipeline is filling.
    """
    qT_all, kT_all = qkin
    with tc.tile_pool(name="at_in", bufs=1, side="right") as ip, \
         tc.tile_pool(name="at_tmp", bufs=2) as tp, \
         tc.tile_pool(name="at_row", bufs=3) as rp, \
         tc.tile_pool(name="at_psq", bufs=2, space="PSUM") as psq, \
         tc.tile_pool(name="at_psA", bufs=2, space="PSUM") as psA, \
         tc.tile_pool(name="at_ps1", bufs=2, space="PSUM") as ps1:
        # b1 queries repacked contiguous: qb1[:, tb] = global b1 blocks
        # (2tb, 2tb+1) = a2a1 slots (7-2tb, 6-2tb) cols OB:OT
        qb1 = ip.tile([128, 4, 512], BF16)
        for tb in range(4):
            nc.gpsimd.dma_start(qb1[:, tb, 0:256],
                                qT_all[:, 7 - 2 * tb, OB:OT])
            nc.gpsimd.dma_start(qb1[:, tb, 256:512],
                                qT_all[:, 6 - 2 * tb, OB:OT])
        v_all = ip.tile([128, R, 4, 256], BF16)
        for r in range(R):
            nc.gpsimd.dma_start(
                v_all[:, r],
                a2av_out[r].rearrange("(p tc) -> p tc", p=128))
        # beta' = exp(Fcum + dmax) broadcast for every (b, query block);
        # bf16: plenty of exponent range, and the mantissa error largely
        # cancels between beta' in A*beta and 1/beta inside gamma
        bbc = ip.tile([128, 2, S], BF16)
        for b in range(2):
            brow = ip.tile([1, S], BF16, name=f"brow{b}")
            for tb in range(4):
                seg = ds(tb * 512, 512)
                nc.scalar.activation(brow[:, seg], Fcum[b][:, seg], AF.Exp,
                                     bias=dm[:, b, tb:tb + 1])
            for tb in range(4):
                seg = ds(tb * 512, 512)
                nc.gpsimd.partition_broadcast(bbc[:, b, seg], brow[:, seg])

        def g_loop(b, tb):
            A0 = psA.tile([128, 512], FP32, tag="A0")
            A1 = psA.tile([128, 512], FP32, tag="A1")
            cpacc = rp.tile([128, 512], BF16, tag="cpacc")
            nst = 4 * tb + 4
            for g in range(nst):
                src = (g // 2) if b == 0 else (7 - g // 2)
                co = (g % 2) * 128 + b * OB       # col offset in payload
                qk = psq.tile([128, 512], FP32, tag="qk")
                if b == 0:
                    nc.tensor.matmul(qk[:], kT_all[:, src, ds(co, 128)],
                                     qT_all[:, ds(2 * tb, 2), 0:OB],
                                     start=True, stop=True)
                else:
                    nc.tensor.matmul(qk[:], kT_all[:, src, ds(co, 128)],
                                     qb1[:, tb], start=True, stop=True)
                cp = tp.tile([128, 512], BF16, tag="cp")
                av = alpha[:, b, tb, g:g + 1]
                if g >= 4 * tb:
                    kk = g - 4 * tb
                    nc.vector.scalar_tensor_tensor(
                        cp[:], qk[:], av, strip[:, ds((3 - kk) * 128, 512)],
                        AL.mult, AL.mult)
                elif g % 2 == 0:
                    # split PSUM evacuation across engines: scalar ACT
                    # Copy with per-partition scale does cp = qk * alpha
                    nc.scalar.activation(cp[:], qk[:], AF.Copy, scale=av)
                else:
                    nc.vector.tensor_scalar_mul(cp[:], qk[:], av)
                # normalizer accumulates on the DVE (no per-g PE pass)
                if g == 0:
                    nc.vector.tensor_copy(cpacc[:], cp[:])
                else:
                    nc.vector.tensor_tensor(cpacc[:], cpacc[:], cp[:], AL.add)
                vi = 2 * b + (g % 2)
                nc.tensor.matmul(A0[:], v_all[:, src, vi, ds(0, 128)], cp[:],
                                 start=(g == 0), stop=(g == nst - 1))
                nc.tensor.matmul(A1[:], v_all[:, src, vi, ds(128, 128)], cp[:],
                                 start=(g == 0), stop=(g == nst - 1))
            return A0, A1, cpacc

        def tail(b, tb, A0, A1, cpacc):
            # --- un-shift per query BEFORE squaring so fp32 FTZ never
            # zeroes the strong-decay heads: beta' = exp(Fcum + dmax);
            # A_true = A_s*beta'; gamma = rsqrt(ssq_true/dv + EPS*n2);
            # n2 = max((n_s*beta')^2, 1); h_norm = (A_s*beta') * gamma
            seg = ds(tb * 512, 512)
            bseg = bbc[:, b, seg]
            rows = ps1.tile([64, 512], FP32, tag="rows")
            nc.tensor.matmul(rows[0:1, :], onesb[:], cpacc[:],
                             start=True, stop=True, skip_group_check=True)
            At0 = tp.tile([128, 512], BF16, tag="At0")
            nc.vector.tensor_tensor(At0[:], A0[:], bseg, AL.mult)
            At1 = tp.tile([128, 512], BF16, tag="At1")
            nc.vector.tensor_tensor(At1[:], A1[:], bseg, AL.mult)
            asq = tp.tile([128, 512], BF16, tag="asq")
            nc.scalar.activation(asq[:], At0[:], AF.Square)
            asq1 = tp.tile([128, 512], BF16, tag="asq1")
            nc.scalar.activation(asq1[:], At1[:], AF.Square)
            nc.vector.tensor_tensor(asq[:], asq[:], asq1[:], AL.add)
            nc.tensor.matmul(rows[32:33, :], onesb[:], asq[:],
                             start=True, stop=True, skip_group_check=True)
            n2 = rp.tile([1, 512], FP32, tag="n2")
            nc.vector.tensor_tensor(n2[:], rows[0:1, :], bbc[0:1, b, seg],
                                    AL.mult)
            nc.vector.scalar_tensor_tensor(n2[:], n2[:], 1.0, n2[:],
                                           AL.mult, AL.mult)
            nc.vector.tensor_scalar_max(n2[:], n2[:], 1.0)
            t1 = rp.tile([1, 512], FP32, tag="t1")
            nc.vector.tensor_scalar_mul(t1[:], rows[32:33, :], 1.0 / dv)
            nc.vector.scalar_tensor_tensor(t1[:], n2[:], EPS, t1[:],
                                           AL.mult, AL.add)
            nc.scalar.activation(t1[:], t1[:], AF.Ln)
            nc.scalar.activation(t1[:], t1[:], AF.Exp, scale=-0.5)
            sb = rp.tile([128, 512], FP32, tag="sbb")
            nc.gpsimd.partition_broadcast(sb[:], t1[:])
            # hn = At * gamma (At = A*beta' already in SBUF) -- A0/A1's
            # PSUM bufs were freed by At0/At1
            hn0 = rp.tile([128, 512], BF16, tag="hn0")
            hn1 = rp.tile([128, 512], BF16, tag="hn1")
            nc.vector.tensor_tensor(hn0[:], At0[:], sb[:], AL.mult)
            nc.vector.tensor_tensor(hn1[:], At1[:], sb[:], AL.mult)
            # payload: token block 2tb+hf -> dest owner core
            for hf in range(2):
                gblk = 2 * tb + hf
                dest = gblk if b == 0 else 7 - gblk
                pay = a2ah_in[b][dest].rearrange("(p h t) -> p h t",
                                                 p=128, h=2)
                nc.sync.dma_start(pay[:, 0, :], hn0[:, ds(hf * 256, 256)])
                nc.sync.dma_start(pay[:, 1, :], hn1[:, ds(hf * 256, 256)])

        def fire_h(bb):
            nc.gpsimd.collective_compute(
                "AllToAll", AL.bypass, replica_groups=rg,
                ins=[a2ah_in[bb][:]], outs=[a2ah_out[bb][:]])
            # immediate reload of the exchanged h (data-gated on the
            # collective; the b1 payloads behind it on the sync queue
            # have later data-ready times anyway)
            nc.sync.dma_start(
                houts[bb][:].rearrange("p (r h) t -> p r h t", h=2),
                a2ah_out[bb].rearrange("r (p h t) -> p r h t", p=128, h=2))

        pend = None
        first_tail = True
        for b in range(2):
            for tb in range(4):
                res = g_loop(b, tb)
                if pend is not None:
                    tail(*pend[0])
                    if first_tail:
                        # big-weight prefetches: scalar queue, data-gated
                        # behind the first tail's chain so they fire after
                        # og's weight stream is done
                        prefetch()
                        first_tail = False
                    if pend[1] is not None:
                        fire_h(pend[1])
                pend = ((b, tb) + res, b if tb == 3 else None)
        tail(*pend[0])
        fire_h(1)


def _post(nc, tc, sigog, ox1_d, onesb, ones1f, wout_d, woh0, houts, x1s, h2):
    """h_out = sig(og)*h_norm; mix = h_out @ w_out batch-major; x1 = x +
    mix; norm2 -> h2 (stays in SBUF for the token-local MLP; no
    collective).  houts and x1s were loaded during attention."""
    with tc.tile_pool(name="po_w1", bufs=1) as w1p, \
         tc.tile_pool(name="po_tmp", bufs=1) as tp, \
         tc.tile_pool(name="po_ps", bufs=3, space="PSUM") as ps, \
         tc.tile_pool(name="po_ps1", bufs=1, space="PSUM") as ps1:
        # second wout half in two tiles on the scalar queue (the sync
        # queue is still draining attention payloads at post start)
        woh1a = w1p.tile([128, NK, 512], BF16, name="woh1a")
        nc.scalar.dma_start(woh1a[:], wout_d[1][:, :, 0:512])
        woh1b = w1p.tile([128, NK, 512], BF16, name="woh1b")
        nc.scalar.dma_start(woh1b[:], wout_d[1][:, :, 512:1024])
        for b in range(2):
            tc_ = ds(b * OB, OB)
            nc.vector.tensor_tensor(houts[b][:], sigog[:, :, tc_],
                                    houts[b][:], AL.mult)
        for b in range(2):
            tc_ = ds(b * OB, OB)
            hout, x1 = houts[b], x1s[b]
            # norm2 ssq accumulates on the DVE inside the wout loop
            ssqa = tp.tile([128, OB], FP32, tag=f"ssqa{b}")
            for m in range(NK):
                woh = woh0 if m < 8 else (woh1a if m < 12 else woh1b)
                mo = (m % 8 if m < 8 else m % 4) * 128
                mix = ps.tile([128, OB], FP32, tag="mix")
                for kt in range(NK):
                    nc.tensor.matmul(mix[:], woh[:, kt, ds(mo, 128)],
                                     hout[:, kt],
                                     start=(kt == 0), stop=(kt == NK - 1))
                nc.vector.tensor_tensor(x1[:, m], x1[:, m], mix[:], AL.add)
                sq = tp.tile([128, OB], BF16, tag="sq2")
                nc.scalar.activation(sq[:], x1[:, m], AF.Square)
                if m == 0:
                    nc.vector.tensor_copy(ssqa[:], sq[:])
                else:
                    nc.vector.tensor_tensor(ssqa[:], ssqa[:], sq[:], AL.add)
            # gpsimd queue: keeps the sync queue free for MLP weights
            nc.gpsimd.dma_start(ox1_d[:, :, tc_], x1[:])
            sqb = tp.tile([128, OB], BF16, tag="sqb")
            nc.vector.tensor_copy(sqb[:], ssqa[:])
            ssq_ps = ps1.tile([1, OB], FP32, tag="ssq2")
            nc.tensor.matmul(ssq_ps[:], onesb[:], sqb[:],
                             start=True, stop=True)
            scl = tp.tile([1, OB], FP32, tag="scl2")
            nc.vector.tensor_scalar(scl[:], ssq_ps[:], 1.0 / D, EPS,
                                    AL.mult, AL.add)
            nc.scalar.activation(scl[:], scl[:], AF.Ln)
            nc.scalar.activation(scl[:], scl[:], AF.Exp, scale=-0.5)
            sc_ps = ps1.tile([128, OB], FP32, tag="sc2")
            nc.tensor.matmul(sc_ps[:], ones1f[0:1, :], scl[:],
                             start=True, stop=True)
            for kt in range(NK):
                nc.vector.tensor_tensor(h2[:, kt, tc_], x1[:, kt], sc_ps[:],
                                        AL.mult)


def _mlp_phase(nc, tc, wg_d, wu_d, wd_d, h2, oy_d):
    """Token-local full-FF SwiGLU for this core's own 512 tokens.

    The FF dimension is streamed in 16 chunks of 512 (double-buffered
    weight tiles; the ~6.3MB/chunk DMA hides under ~55us of matmul), and
    the down-projection partials accumulate in SBUF fp32.  No AllGather,
    no cross-core MLP output reduction."""
    NCH = 16
    with tc.tile_pool(name="ml_w", bufs=2) as wp, \
         tc.tile_pool(name="ml_acc", bufs=1) as acp, \
         tc.tile_pool(name="ml_tmp", bufs=2) as tp, \
         tc.tile_pool(name="ml_ps", bufs=1, space="PSUM") as ps, \
         tc.tile_pool(name="ml_psgu", bufs=2, space="PSUM") as psgu:
        yacc = acp.tile([128, 4, D], FP32)
        for c in range(NCH):
            wgc = wp.tile([128, NK, 512], BF16, tag="wg")
            nc.sync.dma_start(wgc[:], wg_d[c])
            wuc = wp.tile([128, NK, 512], BF16, tag="wu")
            nc.sync.dma_start(wuc[:], wu_d[c])
            wdc = wp.tile([128, 4, D], BF16, tag="wd")
            nc.sync.dma_start(wdc[:], wd_d[c])
            aa = tp.tile([128, 4, OT], BF16, tag="aa")
            for mf in range(4):
                gps = psgu.tile([128, OT], FP32, tag="g")
                for kt in range(NK):
                    nc.tensor.matmul(gps[:], wgc[:, kt, ds(mf * 128, 128)],
                                     h2[:, kt],
                                     start=(kt == 0), stop=(kt == NK - 1))
                ga = tp.tile([128, OT], BF16, tag="ga")
                nc.scalar.activation(ga[:], gps[:], AF.Silu)
                ups = psgu.tile([128, OT], FP32, tag="u")
                for kt in range(NK):
                    nc.tensor.matmul(ups[:], wuc[:, kt, ds(mf * 128, 128)],
                                     h2[:, kt],
                                     start=(kt == 0), stop=(kt == NK - 1))
                nc.vector.tensor_tensor(aa[:, mf], ups[:], ga[:], AL.mult)
            for tt in range(4):
                opss = [ps.tile([128, 512], FP32, tag=f"o{nb}", name=f"o{nb}")
                        for nb in range(4)]
                for kt in range(4):
                    for nb in range(4):
                        nc.tensor.matmul(opss[nb][:],
                                         aa[:, kt, ds(tt * 128, 128)],
                                         wdc[:, kt, ds(nb * 512, 512)],
                                         start=(kt == 0), stop=(kt == 3),
                                         skip_group_check=True)
                for nb in range(4):
                    ysl = yacc[:, tt, ds(nb * 512, 512)]
                    if c == 0:
                        nc.vector.tensor_copy(ysl, opss[nb][:])
                    else:
                        nc.vector.tensor_tensor(ysl, ysl, opss[nb][:], AL.add)
                if c == NCH - 1:
                    # store each token tile as soon as its accumulation
                    # closes: drains the output during the last chunk
                    yb = tp.tile([128, D], BF16, tag="yb")
                    nc.vector.tensor_copy(yb[:], yacc[:, tt])
                    nc.sync.dma_start(oy_d[:, tt], yb[:])


def _build():
    nc = bacc.Bacc(num_devices=R)
    rg = [list(range(R))]

    xT_d = nc.dram_tensor("xT", [128, NK, OT], FP32, kind="ExternalInput")
    xb_d = nc.dram_tensor("xb", [128, NK, OT], BF16, kind="ExternalInput")
    wcat_d = nc.dram_tensor("wcat", [NM, 128, NK, 128], BF16,
                            kind="ExternalInput")
    wv_d = nc.dram_tensor("wv", [4, 128, NK, 512], BF16, kind="ExternalInput")
    b16_d = nc.dram_tensor("b16", [16, 1], FP32, kind="ExternalInput")
    wout_d = nc.dram_tensor("wout", [2, 128, NK, 1024], BF16,
                            kind="ExternalInput")
    # full (unsharded) MLP weights, tiled in 16 ff-chunks of 512
    wg_d = nc.dram_tensor("wg", [16, 128, NK, 512], BF16,
                          kind="ExternalInput")
    wu_d = nc.dram_tensor("wu", [16, 128, NK, 512], BF16,
                          kind="ExternalInput")
    wd_d = nc.dram_tensor("wd", [16, 128, 4, D], BF16, kind="ExternalInput")
    strip_d = nc.dram_tensor("strip", [128, 896], BF16, kind="ExternalInput")
    ones1f_d = nc.dram_tensor("ones1f", [65, 128], FP32, kind="ExternalInput")
    onesb_d = nc.dram_tensor("onesb", [128, 1], BF16, kind="ExternalInput")

    ox1_d = nc.dram_tensor("out_x1", [128, NK, OT], FP32,
                           kind="ExternalOutput")
    oy_d = nc.dram_tensor("out_y", [128, 4, D], BF16, kind="ExternalOutput")

    a2a1_in = nc.dram_tensor("a2a1_in", [R, PAY1], BF16)
    a2a1_out = nc.dram_tensor("a2a1_out", [R, PAY1], BF16)
    a2av_in = nc.dram_tensor("a2av_in", [R, PAYV], BF16)
    a2av_out = nc.dram_tensor("a2av_out", [R, PAYV], BF16)
    ag_g_in = nc.dram_tensor("ag_g_in", [R, 2, OT], FP32)
    dscr = [nc.dram_tensor(f"dscr{b}", [S], FP32) for b in range(2)]
    ag_g_out = nc.dram_tensor("ag_g_out", [R, 2, OT], FP32)
    a2ah_in = [nc.dram_tensor(f"a2ah_in{b}", [R, PAYH], BF16) for b in range(2)]
    a2ah_out = [nc.dram_tensor(f"a2ah_out{b}", [R, PAYH], BF16)
                for b in range(2)]

    with TileContext(nc) as tc:
        with tc.tile_pool(name="glob", bufs=1) as gp:
            # small matmul constants first: the opening ssq matmul needs
            # onesb, and strip (0.22MB) is only read in attention
            onesb = gp.tile([128, 1], BF16)
            nc.sync.dma_start(onesb[:], onesb_d[:])
            ones1f = gp.tile([65, 128], FP32)
            nc.sync.dma_start(ones1f[:], ones1f_d[:])
            b16 = gp.tile([16, 1], FP32)
            nc.sync.dma_start(b16[:], b16_d[:])
            strip = gp.tile([128, 896], BF16)
            nc.sync.dma_start(strip[:], strip_d[:])
            sigog = gp.tile([128, NK, OT], BF16)
            # Fcum lives on the right-side stack: released after attention
            fc_es = ExitStack()
            fcp = fc_es.enter_context(
                tc.tile_pool(name="fcum", bufs=1, side="right"))
            Fcum = [fcp.tile([1, S], FP32, name=f"Fcum{b}") for b in range(2)]
            dm = gp.tile([1, 2, 4], FP32)
            negdm = gp.tile([1, 2, 4], FP32)
            abias = gp.tile([128, 2, 4], FP32)
            alpha = gp.tile([128, 2, 4, 16], FP32)

            # q/k attention-input tiles on the right stack from mixer
            # start: their reloads fire mid-mixer, right after the a2a1
            # collective (b1 queries repacked contiguous: qb1[:, tb] =
            # global b1 blocks (2tb, 2tb+1) = slots (7-2tb, 6-2tb))
            qk_es = ExitStack()
            qkp = qk_es.enter_context(
                tc.tile_pool(name="at_qk", bufs=1, side="right"))
            qkin = (qkp.tile([128, R, OT], BF16, name="qT_all"),
                    qkp.tile([128, R, OT], BF16, name="kT_all"))
            with tc.tile_pool(name="mixp", bufs=1) as mp:
                irow_g = [mp.tile([1, S], FP32, name=f"irow{b}")
                          for b in range(2)]
                dcol = [mp.tile([128, 16], FP32, name=f"dcol{b}")
                        for b in range(2)]
                # bf16 x copy in 4 separate chunk tiles: whole-tile DMA
                # dependencies mean one big tile would serialize on the
                # full load; separate tiles let ssq start on chunk 0
                xbs = []
                for ch in range(4):
                    xc = mp.tile([128, 4, OT], BF16, name=f"xb{ch}")
                    nc.scalar.dma_start(xc[:], xb_d[:, ds(ch * 4, 4)])
                    xbs.append(xc)
                _mixer(nc, tc, xbs, wcat_d, wv_d, b16, sigog, a2a1_in,
                       a2a1_out, a2av_in, a2av_out, ag_g_in, ag_g_out, rg,
                       onesb, ones1f, Fcum, dcol, irow_g, dscr, dm, negdm,
                       abias, alpha, qkin)

            h2_es = ExitStack()
            with tc.tile_pool(name="po_w", bufs=1) as wop, \
                 tc.tile_pool(name="po_h", bufs=1) as hp, \
                 tc.tile_pool(name="po_x1", bufs=2) as xp:
                woh0 = wop.tile([128, NK, 1024], BF16, name="woh0")
                houts = [hp.tile([128, NK, OB], BF16, name=f"hout{b}")
                         for b in range(2)]
                x1s = [xp.tile([128, NK, OB], FP32, tag="x1",
                               name=f"x1_{b}") for b in range(2)]

                def prefetch():
                    nc.scalar.dma_start(woh0[:], wout_d[0])
                    for b in range(2):
                        for ch in range(4):
                            nc.scalar.dma_start(
                                x1s[b][:, ds(ch * 4, 4)],
                                xT_d[:, ds(ch * 4, 4), ds(b * OB, OB)])

                _attention(nc, tc, a2a1_out, a2av_out, strip, Fcum, dm,
                           alpha, onesb, a2ah_in, a2ah_out, rg, prefetch,
                           houts, qkin)
                qk_es.close()
                fc_es.close()
                # h2 (norm2 output) stays in SBUF for the token-local
                # MLP; right-side stack so it outlives the post pools
                h2p = h2_es.enter_context(
                    tc.tile_pool(name="po_h2", bufs=1, side="right"))
                h2 = h2p.tile([128, NK, OT], BF16, name="h2")
                _post(nc, tc, sigog, ox1_d, onesb, ones1f, wout_d, woh0,
                      houts, x1s, h2)
            _mlp_phase(nc, tc, wg_d, wu_d, wd_d, h2, oy_d)
            h2_es.close()

    nc.finalize()
    return nc


_NC_CACHE = None


def _tile_kt(w):
    """[D, F] -> [128, NK, F] so each partition's stream is contiguous."""
    return np.ascontiguousarray(
        w.reshape(NK, 128, w.shape[1]).transpose(1, 0, 2))


def kernel(x, norm1_w, wq, wk, wv, w_ig, b_ig, w_fg, b_fg, w_og, mh_w,
           w_out, norm2_w, w_gate, w_up, w_down):
    global _NC_CACHE
    x = np.asarray(x, np.float32)
    n1 = np.asarray(norm1_w, np.float32)
    n2 = np.asarray(norm2_w, np.float32)
    mh = np.asarray(mh_w, np.float32)

    wif = np.empty((D, 2 * H), np.float32)
    wif[:, 0::2] = np.asarray(w_ig)
    wif[:, 1::2] = np.asarray(w_fg)
    b16v = np.empty((16, 1), np.float32)
    b16v[0::2, 0] = -2.0 * np.asarray(b_ig) / CAP
    b16v[1::2, 0] = -2.0 * np.asarray(b_fg) / CAP

    wcat = (np.concatenate([np.asarray(wq), np.asarray(wk), np.asarray(w_og), wif],
                           axis=1) * n1[:, None]).astype(bf16)
    wcat_p = np.zeros((D, NM * 128), bf16)
    wcat_p[:, :wcat.shape[1]] = wcat
    # [m, p, kt, c]: per (m-tile, partition) rows fully contiguous
    wcat_t = np.ascontiguousarray(
        wcat_p.reshape(NK, 128, NM, 128).transpose(2, 1, 0, 3))
    wv_b = (np.asarray(wv) * n1[:, None]).astype(bf16)
    wv_t = np.ascontiguousarray(
        wv_b.reshape(NK, 128, 4, 512).transpose(2, 1, 0, 3))
    wout_f = (np.asarray(w_out) * mh[:, None]).astype(bf16)
    wout_t = np.ascontiguousarray(
        wout_f.reshape(NK, 128, 2, 1024).transpose(2, 1, 0, 3))
    # full MLP weights, ff-chunked: [16, 128, NK, 512] / [16, 128, 4, D]
    wg_t = np.ascontiguousarray(
        (np.asarray(w_gate) * n2[:, None]).astype(bf16)
        .reshape(NK, 128, 16, 512).transpose(2, 1, 0, 3))
    wu_t = np.ascontiguousarray(
        (np.asarray(w_up) * n2[:, None]).astype(bf16)
        .reshape(NK, 128, 16, 512).transpose(2, 1, 0, 3))
    wd_t = np.ascontiguousarray(
        np.asarray(w_down).astype(bf16)
        .reshape(16, 4, 128, D).transpose(0, 2, 1, 3))

    i_idx = np.arange(128)[:, None]
    c_idx = np.arange(896)[None, :]
    strip = ((c_idx - i_idx) >= 384).astype(bf16)
    ones1f = np.ones((65, 128), np.float32)
    onesb = np.ones((128, 1), bf16)

    in_maps = []
    for c in range(R):
        s0 = slice(OB * c, OB * (c + 1))
        s1 = slice(OB * (7 - c), OB * (8 - c))
        xT = np.ascontiguousarray(
            np.concatenate([x[0, s0].T, x[1, s1].T], axis=1)).astype(np.float32)
        xT_t = _tile_kt(xT)
        in_maps.append({
            "xT": xT_t, "xb": xT_t.astype(bf16), "wcat": wcat_t, "wv": wv_t,
            "b16": b16v, "wout": wout_t,
            "wg": wg_t, "wu": wu_t, "wd": wd_t,
            "strip": strip, "ones1f": ones1f, "onesb": onesb,
        })

    if _NC_CACHE is None:
        _NC_CACHE = _build()
    res = run_bass_kernel_spmd(_NC_CACHE, in_maps, core_ids=list(range(R)))

    out = np.zeros((B, S, D), np.float32)
    for c in range(R):
        x1T = np.asarray(res.results[c]["out_x1"]).astype(np.float32)
        x1T = x1T.transpose(1, 0, 2).reshape(D, OT)
        y = np.asarray(res.results[c]["out_y"]).astype(np.float32)
        s0 = slice(OB * c, OB * (c + 1))
        s1 = slice(OB * (7 - c), OB * (8 - c))
        out[0, s0] = x1T[:, :OB].T
        out[1, s1] = x1T[:, OB:].T
        out[0, s0] += y[:, 0:2].transpose(1, 0, 2).reshape(OB, D)
        out[1, s1] += y[:, 2:4].transpose(1, 0, 2).reshape(OB, D)
    return out
